# revision 24
# baseline (speedup 1.0000x reference)
"""Bahdanau attention kernel for Trainium2, data-parallel over batch on 8 NeuronCores.

Per core (B_local = 4 batches):
  pass 1: encT tiles via PE matmul against identity (enc_block^T @ I),
          enc_att^T[a,s] = W_enc @ enc[s,:]^T  (bf16 matmul, fp32 accum)
          T = tanh(enc_att^T + dec_att^T[a] bias)  (ACT, fused PSUM->SBUF)
          scores[s] = v . T[:,s]                  (M=1 matmul)
  softmax: u = exp(scores) with fused partial sums (accum_out)
  pass 2: context[he] = sum_s u[s] * enc[s,he], scaled by 1/sum at the end

enc streams in once as fp32 (cast to bf16 in-flight by SWDGE), stays SBUF-resident
in natural [s,h] layout for pass 2; [h,s] tiles for pass 1 are produced on the PE
(plain matmuls with identity rhs - keeps the HAM clock warm) and evacuated
PSUM->SBUF by ACT/DVE alternately.  The tanh/v-dot/exp epilogue of each s-tile is
emitted 1-2 att-groups late so the strict PE FIFO never stalls on ACT latency.
"""

import os
import numpy as np
import ml_dtypes

import concourse.bacc as bacc
import concourse.tile as tile
import concourse.mybir as mybir
from concourse import bass_utils

BF16 = mybir.dt.bfloat16
F32 = mybir.dt.float32
AF = mybir.ActivationFunctionType
bf16 = ml_dtypes.bfloat16

N_CORES = 8
S, B, HE, HD, A = 2048, 32, 1024, 1024, 512


class Cfg:
    def __init__(self, S, BL, He, A, n_cores):
        assert S % 512 == 0 and He % 128 == 0 and A % 128 == 0
        self.S, self.BL, self.He, self.A, self.n_cores = S, BL, He, A, n_cores
        self.SC = S // 128     # s chunks of 128
        self.ST = S // 512     # s tiles of 512
        self.HC = He // 128    # h chunks
        self.AC = A // 128     # a chunks
        self.HH = 512 if He % 512 == 0 else He  # context free-dim chunk


def emit(tc, cfg, outs, ins):
    nc = tc.nc
    S, BL, He, A = cfg.S, cfg.BL, cfg.He, cfg.A
    SC, ST, HC, AC, HH = cfg.SC, cfg.ST, cfg.HC, cfg.AC, cfg.HH
    ctx_out, w_out = outs
    enc, wencT, wdecT, decT, vv, eye1, eye128 = ins

    from contextlib import ExitStack
    with ExitStack() as ctx:
        pw = ctx.enter_context(tc.tile_pool(name="w", bufs=1))
        penc = ctx.enter_context(tc.tile_pool(name="encres", bufs=1))
        pencT = ctx.enter_context(tc.tile_pool(name="encT", bufs=3))
        pT = ctx.enter_context(tc.tile_pool(name="T", bufs=4))
        pu = ctx.enter_context(tc.tile_pool(name="u", bufs=2))
        pctx = ctx.enter_context(tc.tile_pool(name="ctxp", bufs=2))
        pwT = ctx.enter_context(tc.tile_pool(name="wT", bufs=2))
        pmisc = ctx.enter_context(tc.tile_pool(name="misc", bufs=8))
        psA = ctx.enter_context(tc.tile_pool(name="psA", bufs=3, space="PSUM"))
        psT = ctx.enter_context(tc.tile_pool(name="psT", bufs=2, space="PSUM"))
        psS = ctx.enter_context(tc.tile_pool(name="psS", bufs=1, space="PSUM"))
        psM = ctx.enter_context(tc.tile_pool(name="psM", bufs=2, space="PSUM"))

        # ---- setup loads; tiny constants on the sync queue ----
        eye128_sb = pw.tile([128, 128], BF16, tag="eye128")
        nc.sync.dma_start(eye128_sb[:], eye128[:])
        eye_sb = pw.tile([1, 1], F32, tag="eye")
        nc.sync.dma_start(eye_sb[:], eye1[:])
        dec_attT = pw.tile([128, AC, BL], F32, tag="dec_attT")

        # ---- enc resident cast-loads (SWDGE, 16 queues) interleaved with the
        # parameter loads so nothing gates the PE ramp-up ----
        # enc rows r = s*BL + b, s = ((st*4 + cq)*128 + p)
        enc_r = enc.rearrange("(st cq p b) h -> b st p cq h", st=ST, cq=4, p=128, b=BL)
        enc_bf = {}

        def load_tile(b, st):
            t = penc.tile([128, 4, He], BF16, tag=f"encres_{b}_{st}",
                          name=f"encres_{b}_{st}")
            # sub-tile loads so the first transposes start after 512 KB,
            # not after the whole 2 MiB tile
            for q in range(4):
                nc.gpsimd.dma_start(t[:, q, :], enc_r[b, st, :, q, :])
            enc_bf[(b, st)] = t

        # HAM warmup: keep the PE busy from the moment eye128 lands so the
        # clock gate is at 2.4 GHz when the first real tiles arrive, and the
        # initial load-wait windows hold work instead of idling
        warm_ps = psM.tile([128, 128], F32, tag="m", name="warm1")
        for k in range(24):
            nc.tensor.matmul(warm_ps[:], eye128_sb[:], eye128_sb[:],
                             start=(k == 0), stop=(k == 23))

        load_tile(0, 0)
        wencT_sb = pw.tile([128, HC, A], BF16, tag="wencT")
        nc.gpsimd.dma_start(wencT_sb[:], wencT[:])
        # wdecT parks in the encT pool slot; freed for encT tiles after dec matmuls
        wdecT_sb = pencT.tile([128, HC, A], BF16, tag="encT")
        nc.gpsimd.dma_start(wdecT_sb[:], wdecT[:])
        decT_sb = pw.tile([128, HC, BL], BF16, tag="decT")
        nc.gpsimd.dma_start(decT_sb[:], decT[:])
        v_sb = pw.tile([128, AC, 1], BF16, tag="v")
        nc.gpsimd.dma_start(v_sb[:], vv[:])
        load_tile(0, 1)
        for b in range(BL):
            for st in range(ST):
                if (b, st) not in enc_bf:
                    load_tile(b, st)

        # ---- pipelined main loop over the 16 (b, st) tiles ----
        tiles = [(b, st) for b in range(BL) for st in range(ST)]
        state = {}      # per-b tiles (u, partials)
        deferred = []   # closures to emit at the next tile's flush point

        def b_tail(b):
            u_b, part_b = state.pop(b)
            ssum = pmisc.tile([1, 1], F32, tag="ssum")
            nc.vector.reduce_sum(ssum[:], part_b[:], axis=mybir.AxisListType.X)
            rinv = pmisc.tile([1, 1], F32, tag="rinv")
            nc.vector.reciprocal(rinv[:], ssum[:])
            # w^T chunks from RAW u (bf16); pass-2 results get scaled by rinv
            wT_b = pwT.tile([128, SC, 1], BF16, tag="wT")
            ctx_sb = pctx.tile([1, He], F32, tag="ctx")
            # interleave the wT build with the first half of pass 2 so the
            # context matmuls start as soon as each wT chunk lands
            cps0 = psM.tile([1, HH], F32, tag="m", name=f"cps0_{b}")
            for c0 in range(0, SC, 4):
                wps = psM.tile([128, 4], F32, tag="m", name=f"wps_{b}_{c0}")
                for k in range(4):
                    c = c0 + k
                    nc.tensor.matmul(wps[:, k:k + 1],
                                     u_b[0:1, c * 128:(c + 1) * 128], eye_sb[:],
                                     is_transpose=True,
                                     start=(k == 0), stop=(k == 3))
                nc.vector.tensor_copy(wT_b[:, c0:c0 + 4, :], wps[:])
                for c in range(c0, c0 + 4):
                    stc, q = c // 4, c % 4
                    nc.tensor.matmul(cps0[:], wT_b[:, c, :],
                                     enc_bf[(b, stc)][:, q, 0:HH],
                                     start=(c == 0), stop=(c == SC - 1))
            nc.scalar.mul(ctx_sb[0:1, 0:HH], cps0[:], rinv[:])
            # normalized weights out (in-place after wT reads; Tile orders the WAR)
            nc.vector.tensor_scalar_mul(u_b[:], u_b[:], rinv[:])
            nc.sync.dma_start(w_out[b:b + 1, :], u_b[:])
            # remaining pass-2 halves
            for hh in range(1, He // HH):
                cps = psM.tile([1, HH], F32, tag="m")
                for c in range(SC):
                    stc, q = c // 4, c % 4
                    nc.tensor.matmul(cps[:], wT_b[:, c, :],
                                     enc_bf[(b, stc)][:, q, hh * HH:(hh + 1) * HH],
                                     start=(c == 0), stop=(c == SC - 1))
                nc.scalar.mul(ctx_sb[0:1, hh * HH:(hh + 1) * HH], cps[:], rinv[:])
            nc.sync.dma_start(ctx_out[b:b + 1, :], ctx_sb[:])

        copy_flip = 0
        for i, (b, st) in enumerate(tiles):
            if st == 0:
                u_b = pu.tile([1, S], F32, tag="u")
                part_b = pmisc.tile([1, ST], F32, tag="part")
                state[b] = (u_b, part_b)
            u_b, part_b = state[b]
            src = enc_bf[(b, st)]

            # phase A: transposes for this tile (PE matmuls vs identity)
            encT_t = pencT.tile([128, HC, 512], BF16, tag="encT")
            for hc in range(HC):
                tps = psT.tile([128, 512], F32, tag="tr")
                for q in range(4):
                    nc.tensor.matmul(tps[:, q * 128:(q + 1) * 128],
                                     src[:, q, hc * 128:(hc + 1) * 128],
                                     eye128_sb[:],
                                     start=(q == 0), stop=(q == 3))
                # 3:5 ACT:DVE split - ACT also carries the tanh stream
                eng_act = (copy_flip % 8) in (0, 3, 6)
                copy_flip += 1
                if eng_act:
                    nc.scalar.copy(encT_t[:, hc, :], tps[:])
                else:
                    nc.vector.tensor_copy(encT_t[:, hc, :], tps[:])

            if i == 0:
                warm2 = psM.tile([128, 128], F32, tag="m", name="warm2")
                for k in range(24):
                    nc.tensor.matmul(warm2[:], eye128_sb[:], eye128_sb[:],
                                     start=(k == 0), stop=(k == 23))

            # phase B: flush previous tile's trailing epilogue
            for fn in deferred:
                fn()
            deferred = []

            # phase C: att groups with staggered tanh / v-dot
            sc_ps = psS.tile([1, 512], F32, tag="sc")
            att_ps, T_t = [None] * AC, [None] * AC

            def mk_att(ac):
                att_ps[ac] = psA.tile([128, 512], F32, tag="att", name=f"att_{i}_{ac}")
                for hc in range(HC):
                    nc.tensor.matmul(att_ps[ac][:],
                                     wencT_sb[:, hc, ac * 128:(ac + 1) * 128],
                                     encT_t[:, hc, :],
                                     start=(hc == 0), stop=(hc == HC - 1))

            def mk_tanh(ac):
                T_t[ac] = pT.tile([128, 512], BF16, tag="T", name=f"T_{i}_{ac}")
                nc.scalar.activation(T_t[ac][:], att_ps[ac][:], AF.Tanh,
                                     bias=dec_attT[:, ac, b:b + 1])

            def mk_vdot(ac):
                nc.tensor.matmul(sc_ps[:], v_sb[:, ac, :], T_t[ac][:],
                                 start=(ac == 0), stop=(ac == AC - 1))

            for ac in range(AC):
                mk_att(ac)
                if i == 0 and ac == 0:
                    # dec_att^T[a, b] (small; emitted after the first att group
                    # so slow parameter loads can never head-block the PE FIFO)
                    for dac in range(AC):
                        dps = psM.tile([128, BL], F32, tag="m")
                        for hc in range(HC):
                            nc.tensor.matmul(dps[:],
                                             wdecT_sb[:, hc, dac * 128:(dac + 1) * 128],
                                             decT_sb[:, hc, :],
                                             start=(hc == 0), stop=(hc == HC - 1))
                        nc.scalar.copy(dec_attT[:, dac, :], dps[:])
                if ac >= 1:
                    mk_tanh(ac - 1)
                if ac >= 2:
                    mk_vdot(ac - 2)
            mk_tanh(AC - 1)

            def tail(sc_ps=sc_ps, u_b=u_b, part_b=part_b, b=b, st=st,
                     mk_vdot=mk_vdot):
                mk_vdot(AC - 2)
                mk_vdot(AC - 1)
                nc.scalar.activation(u_b[0:1, st * 512:(st + 1) * 512], sc_ps[:],
                                     AF.Exp, accum_out=part_b[0:1, st:st + 1])
                if st == ST - 1:
                    b_tail(b)
            deferred.append(tail)

        for fn in deferred:
            fn()


def build(cfg):
    nc = bacc.Bacc("TRN2", target_bir_lowering=False, debug=False,
                   enable_asserts=False, num_devices=cfg.n_cores)
    enc = nc.dram_tensor("enc", [cfg.S * cfg.BL, cfg.He], F32, kind="ExternalInput").ap()
    wencT = nc.dram_tensor("wencT", [128, cfg.HC, cfg.A], BF16, kind="ExternalInput").ap()
    wdecT = nc.dram_tensor("wdecT", [128, cfg.HC, cfg.A], BF16, kind="ExternalInput").ap()
    decT = nc.dram_tensor("decT", [128, cfg.HC, cfg.BL], BF16, kind="ExternalInput").ap()
    vv = nc.dram_tensor("vv", [128, cfg.AC, 1], BF16, kind="ExternalInput").ap()
    eye1 = nc.dram_tensor("eye1", [1, 1], F32, kind="ExternalInput").ap()
    eye128 = nc.dram_tensor("eye128", [128, 128], BF16, kind="ExternalInput").ap()
    ctx_out = nc.dram_tensor("ctx", [cfg.BL, cfg.He], F32, kind="ExternalOutput").ap()
    w_out = nc.dram_tensor("wout", [cfg.BL, cfg.S], F32, kind="ExternalOutput").ap()
    with tile.TileContext(nc) as tc:
        emit(tc, cfg, (ctx_out, w_out), (enc, wencT, wdecT, decT, vv, eye1, eye128))
    nc.compile()
    return nc


_cache = {}


def _get_module():
    if "nc" not in _cache:
        _cache["nc"] = build(Cfg(S, B // N_CORES, HE, A, N_CORES))
    return _cache["nc"]


def kernel(dec_out, enc_outs, W_enc, W_dec, att_v):
    dec_out = np.asarray(dec_out, dtype=np.float32)
    enc_outs = np.asarray(enc_outs, dtype=np.float32)
    W_enc = np.asarray(W_enc, dtype=np.float32)
    W_dec = np.asarray(W_dec, dtype=np.float32)
    att_v = np.asarray(att_v, dtype=np.float32)
    assert dec_out.shape == (B, HD) and enc_outs.shape == (S, B, HE)
    assert W_enc.shape == (A, HE) and W_dec.shape == (A, HD) and att_v.shape == (A,)
    nc = _get_module()
    BL = B // N_CORES

    def pack(mT):  # [rows=128*G, cols] -> [128, G, cols] partition-major
        g = mT.shape[0] // 128
        return np.ascontiguousarray(
            mT.reshape(g, 128, -1).transpose(1, 0, 2)).astype(bf16)

    wencT_np = pack(W_enc.T)
    wdecT_np = pack(W_dec.T)
    vv_np = pack(att_v[:, None])
    eye_np = np.ones((1, 1), np.float32)
    eye128_np = np.eye(128).astype(bf16)

    in_maps = []
    for c in range(N_CORES):
        bs = slice(c * BL, (c + 1) * BL)
        in_maps.append({
            "enc": np.ascontiguousarray(enc_outs[:, bs, :]).reshape(S * BL, HE),
            "wencT": wencT_np,
            "wdecT": wdecT_np,
            "decT": pack(dec_out[bs].T),
            "vv": vv_np,
            "eye1": eye_np,
            "eye128": eye128_np,
        })

    trace = bool(int(os.environ.get("BAHDANAU_TRACE", "0")))
    res = bass_utils.run_bass_kernel_spmd(
        nc, in_maps, core_ids=list(range(N_CORES)), trace=trace)
    _cache["last_result"] = res

    context = np.concatenate([res.results[c]["ctx"] for c in range(N_CORES)], axis=0)
    weights = np.concatenate([res.results[c]["wout"] for c in range(N_CORES)], axis=0)
    return context, weights


# revision 25
# speedup vs baseline: 1.0125x; 1.0125x over previous
"""Bahdanau attention kernel for Trainium2, data-parallel over batch on 8 NeuronCores.

Per core (B_local = 4 batches):
  pass 1: encT tiles via PE matmul against identity (enc_block^T @ I),
          enc_att^T[a,s] = W_enc @ enc[s,:]^T  (bf16 matmul, fp32 accum)
          T = tanh(enc_att^T + dec_att^T[a] bias)  (ACT, fused PSUM->SBUF)
          scores[s] = v . T[:,s]                  (M=1 matmul)
  softmax: u = exp(scores) with fused partial sums (accum_out)
  pass 2: context[he] = sum_s u[s] * enc[s,he], scaled by 1/sum at the end

enc streams in once as fp32 (cast to bf16 in-flight by SWDGE), stays SBUF-resident
in natural [s,h] layout for pass 2; [h,s] tiles for pass 1 are produced on the PE
(plain matmuls with identity rhs - keeps the HAM clock warm) and evacuated
PSUM->SBUF by ACT/DVE alternately.  The tanh/v-dot/exp epilogue of each s-tile is
emitted 1-2 att-groups late so the strict PE FIFO never stalls on ACT latency.
"""

import os
import numpy as np
import ml_dtypes

import concourse.bacc as bacc
import concourse.tile as tile
import concourse.mybir as mybir
from concourse import bass_utils

BF16 = mybir.dt.bfloat16
F32 = mybir.dt.float32
AF = mybir.ActivationFunctionType
bf16 = ml_dtypes.bfloat16

N_CORES = 8
S, B, HE, HD, A = 2048, 32, 1024, 1024, 512


class Cfg:
    def __init__(self, S, BL, He, A, n_cores):
        assert S % 512 == 0 and He % 128 == 0 and A % 128 == 0
        self.S, self.BL, self.He, self.A, self.n_cores = S, BL, He, A, n_cores
        self.SC = S // 128     # s chunks of 128
        self.ST = S // 512     # s tiles of 512
        self.HC = He // 128    # h chunks
        self.AC = A // 128     # a chunks
        self.HH = 512 if He % 512 == 0 else He  # context free-dim chunk


def emit(tc, cfg, outs, ins):
    nc = tc.nc
    S, BL, He, A = cfg.S, cfg.BL, cfg.He, cfg.A
    SC, ST, HC, AC, HH = cfg.SC, cfg.ST, cfg.HC, cfg.AC, cfg.HH
    ctx_out, w_out = outs
    enc, wencT, wdecT, decT, vv, eye1, eye128 = ins

    from contextlib import ExitStack
    with ExitStack() as ctx:
        pw = ctx.enter_context(tc.tile_pool(name="w", bufs=1))
        penc = ctx.enter_context(tc.tile_pool(name="encres", bufs=1))
        pencT = ctx.enter_context(tc.tile_pool(name="encT", bufs=3))
        pT = ctx.enter_context(tc.tile_pool(name="T", bufs=4))
        pu = ctx.enter_context(tc.tile_pool(name="u", bufs=2))
        pctx = ctx.enter_context(tc.tile_pool(name="ctxp", bufs=2))
        pwT = ctx.enter_context(tc.tile_pool(name="wT", bufs=2))
        pmisc = ctx.enter_context(tc.tile_pool(name="misc", bufs=8))
        psA = ctx.enter_context(tc.tile_pool(name="psA", bufs=3, space="PSUM"))
        psT = ctx.enter_context(tc.tile_pool(name="psT", bufs=3, space="PSUM"))
        psS = ctx.enter_context(tc.tile_pool(name="psS", bufs=1, space="PSUM"))
        psM = ctx.enter_context(tc.tile_pool(name="psM", bufs=1, space="PSUM"))

        # ---- setup loads; tiny constants on the sync queue ----
        eye128_sb = pw.tile([128, 128], BF16, tag="eye128")
        nc.sync.dma_start(eye128_sb[:], eye128[:])
        eye_sb = pw.tile([1, 1], F32, tag="eye")
        nc.sync.dma_start(eye_sb[:], eye1[:])
        dec_attT = pw.tile([128, AC, BL], F32, tag="dec_attT")

        # ---- enc resident cast-loads (SWDGE, 16 queues) interleaved with the
        # parameter loads so nothing gates the PE ramp-up ----
        # enc rows r = s*BL + b, s = ((st*4 + cq)*128 + p)
        enc_r = enc.rearrange("(st cq p b) h -> b st p cq h", st=ST, cq=4, p=128, b=BL)
        enc_bf = {}

        def load_tile(b, st):
            t = penc.tile([128, 4, He], BF16, tag=f"encres_{b}_{st}",
                          name=f"encres_{b}_{st}")
            # sub-tile loads so the first transposes start after 512 KB,
            # not after the whole 2 MiB tile
            for q in range(4):
                nc.gpsimd.dma_start(t[:, q, :], enc_r[b, st, :, q, :])
            enc_bf[(b, st)] = t

        # HAM warmup: keep the PE busy from the moment eye128 lands so the
        # clock gate is at 2.4 GHz when the first real tiles arrive, and the
        # initial load-wait windows hold work instead of idling
        warm_ps = psM.tile([128, 128], F32, tag="m", name="warm1")
        for k in range(24):
            nc.tensor.matmul(warm_ps[:], eye128_sb[:], eye128_sb[:],
                             start=(k == 0), stop=(k == 23))

        load_tile(0, 0)
        wencT_sb = pw.tile([128, HC, A], BF16, tag="wencT")
        nc.gpsimd.dma_start(wencT_sb[:], wencT[:])
        # wdecT parks in the encT pool slot; freed for encT tiles after dec matmuls
        wdecT_sb = pencT.tile([128, HC, A], BF16, tag="encT")
        nc.gpsimd.dma_start(wdecT_sb[:], wdecT[:])
        decT_sb = pw.tile([128, HC, BL], BF16, tag="decT")
        nc.gpsimd.dma_start(decT_sb[:], decT[:])
        v_sb = pw.tile([128, AC, 1], BF16, tag="v")
        nc.gpsimd.dma_start(v_sb[:], vv[:])
        load_tile(0, 1)
        for b in range(BL):
            for st in range(ST):
                if (b, st) not in enc_bf:
                    load_tile(b, st)

        # ---- pipelined main loop over the 16 (b, st) tiles ----
        tiles = [(b, st) for b in range(BL) for st in range(ST)]
        state = {}      # per-b tiles (u, partials)
        deferred = []   # closures to emit at the next tile's flush point

        def b_tail(b):
            u_b, part_b = state.pop(b)
            ssum = pmisc.tile([1, 1], F32, tag="ssum")
            nc.vector.reduce_sum(ssum[:], part_b[:], axis=mybir.AxisListType.X)
            rinv = pmisc.tile([1, 1], F32, tag="rinv")
            nc.vector.reciprocal(rinv[:], ssum[:])
            # w^T chunks from RAW u (bf16); pass-2 results get scaled by rinv
            wT_b = pwT.tile([128, SC, 1], BF16, tag="wT")
            ctx_sb = pctx.tile([1, He], F32, tag="ctx")
            for c0 in range(0, SC, 4):
                wps = psM.tile([128, 4], F32, tag="m", name=f"wps_{b}_{c0}")
                for k in range(4):
                    c = c0 + k
                    nc.tensor.matmul(wps[:, k:k + 1],
                                     u_b[0:1, c * 128:(c + 1) * 128], eye_sb[:],
                                     is_transpose=True,
                                     start=(k == 0), stop=(k == 3))
                nc.vector.tensor_copy(wT_b[:, c0:c0 + 4, :], wps[:])
            # normalized weights out (in-place after wT reads; Tile orders the WAR)
            nc.vector.tensor_scalar_mul(u_b[:], u_b[:], rinv[:])
            nc.sync.dma_start(w_out[b:b + 1, :], u_b[:])
            # pass 2
            for hh in range(0, He // HH):
                cps = psM.tile([1, HH], F32, tag="m")
                for c in range(SC):
                    stc, q = c // 4, c % 4
                    nc.tensor.matmul(cps[:], wT_b[:, c, :],
                                     enc_bf[(b, stc)][:, q, hh * HH:(hh + 1) * HH],
                                     start=(c == 0), stop=(c == SC - 1))
                nc.scalar.mul(ctx_sb[0:1, hh * HH:(hh + 1) * HH], cps[:], rinv[:])
            nc.sync.dma_start(ctx_out[b:b + 1, :], ctx_sb[:])

        copy_flip = 0
        for i, (b, st) in enumerate(tiles):
            if st == 0:
                u_b = pu.tile([1, S], F32, tag="u")
                part_b = pmisc.tile([1, ST], F32, tag="part")
                state[b] = (u_b, part_b)
            u_b, part_b = state[b]
            src = enc_bf[(b, st)]

            # phase A: transposes for this tile (PE matmuls vs identity)
            encT_t = pencT.tile([128, HC, 512], BF16, tag="encT")
            for hc in range(HC):
                tps = psT.tile([128, 512], F32, tag="tr")
                for q in range(4):
                    nc.tensor.matmul(tps[:, q * 128:(q + 1) * 128],
                                     src[:, q, hc * 128:(hc + 1) * 128],
                                     eye128_sb[:],
                                     start=(q == 0), stop=(q == 3))
                # 3:5 ACT:DVE split - ACT also carries the tanh stream
                eng_act = (copy_flip % 8) in (0, 3, 6)
                copy_flip += 1
                if eng_act:
                    nc.scalar.copy(encT_t[:, hc, :], tps[:])
                else:
                    nc.vector.tensor_copy(encT_t[:, hc, :], tps[:])

            if i == 0:
                warm2 = psM.tile([128, 128], F32, tag="m", name="warm2")
                for k in range(24):
                    nc.tensor.matmul(warm2[:], eye128_sb[:], eye128_sb[:],
                                     start=(k == 0), stop=(k == 23))

            # phase B: flush previous tile's trailing epilogue
            for fn in deferred:
                fn()
            deferred = []

            # phase C: att groups with staggered tanh / v-dot
            sc_ps = psS.tile([1, 512], F32, tag="sc")
            att_ps, T_t = [None] * AC, [None] * AC

            def mk_att(ac):
                att_ps[ac] = psA.tile([128, 512], F32, tag="att", name=f"att_{i}_{ac}")
                for hc in range(HC):
                    nc.tensor.matmul(att_ps[ac][:],
                                     wencT_sb[:, hc, ac * 128:(ac + 1) * 128],
                                     encT_t[:, hc, :],
                                     start=(hc == 0), stop=(hc == HC - 1))

            def mk_tanh(ac):
                T_t[ac] = pT.tile([128, 512], BF16, tag="T", name=f"T_{i}_{ac}")
                nc.scalar.activation(T_t[ac][:], att_ps[ac][:], AF.Tanh,
                                     bias=dec_attT[:, ac, b:b + 1])

            def mk_vdot(ac):
                nc.tensor.matmul(sc_ps[:], v_sb[:, ac, :], T_t[ac][:],
                                 start=(ac == 0), stop=(ac == AC - 1))

            for ac in range(AC):
                mk_att(ac)
                if i == 0 and ac == 0:
                    # dec_att^T[a, b] (small; emitted after the first att group
                    # so slow parameter loads can never head-block the PE FIFO)
                    for dac in range(AC):
                        dps = psM.tile([128, BL], F32, tag="m")
                        for hc in range(HC):
                            nc.tensor.matmul(dps[:],
                                             wdecT_sb[:, hc, dac * 128:(dac + 1) * 128],
                                             decT_sb[:, hc, :],
                                             start=(hc == 0), stop=(hc == HC - 1))
                        nc.scalar.copy(dec_attT[:, dac, :], dps[:])
                if ac >= 1:
                    mk_tanh(ac - 1)
                if ac >= 2:
                    mk_vdot(ac - 2)
            mk_tanh(AC - 1)

            def tail(sc_ps=sc_ps, u_b=u_b, part_b=part_b, b=b, st=st,
                     mk_vdot=mk_vdot):
                mk_vdot(AC - 2)
                mk_vdot(AC - 1)
                nc.scalar.activation(u_b[0:1, st * 512:(st + 1) * 512], sc_ps[:],
                                     AF.Exp, accum_out=part_b[0:1, st:st + 1])
                if st == ST - 1:
                    b_tail(b)
            deferred.append(tail)

        for fn in deferred:
            fn()


def build(cfg):
    nc = bacc.Bacc("TRN2", target_bir_lowering=False, debug=False,
                   enable_asserts=False, num_devices=cfg.n_cores)
    enc = nc.dram_tensor("enc", [cfg.S * cfg.BL, cfg.He], F32, kind="ExternalInput").ap()
    wencT = nc.dram_tensor("wencT", [128, cfg.HC, cfg.A], BF16, kind="ExternalInput").ap()
    wdecT = nc.dram_tensor("wdecT", [128, cfg.HC, cfg.A], BF16, kind="ExternalInput").ap()
    decT = nc.dram_tensor("decT", [128, cfg.HC, cfg.BL], BF16, kind="ExternalInput").ap()
    vv = nc.dram_tensor("vv", [128, cfg.AC, 1], BF16, kind="ExternalInput").ap()
    eye1 = nc.dram_tensor("eye1", [1, 1], F32, kind="ExternalInput").ap()
    eye128 = nc.dram_tensor("eye128", [128, 128], BF16, kind="ExternalInput").ap()
    ctx_out = nc.dram_tensor("ctx", [cfg.BL, cfg.He], F32, kind="ExternalOutput").ap()
    w_out = nc.dram_tensor("wout", [cfg.BL, cfg.S], F32, kind="ExternalOutput").ap()
    with tile.TileContext(nc) as tc:
        emit(tc, cfg, (ctx_out, w_out), (enc, wencT, wdecT, decT, vv, eye1, eye128))
    nc.compile()
    return nc


_cache = {}


def _get_module():
    if "nc" not in _cache:
        _cache["nc"] = build(Cfg(S, B // N_CORES, HE, A, N_CORES))
    return _cache["nc"]


def kernel(dec_out, enc_outs, W_enc, W_dec, att_v):
    dec_out = np.asarray(dec_out, dtype=np.float32)
    enc_outs = np.asarray(enc_outs, dtype=np.float32)
    W_enc = np.asarray(W_enc, dtype=np.float32)
    W_dec = np.asarray(W_dec, dtype=np.float32)
    att_v = np.asarray(att_v, dtype=np.float32)
    assert dec_out.shape == (B, HD) and enc_outs.shape == (S, B, HE)
    assert W_enc.shape == (A, HE) and W_dec.shape == (A, HD) and att_v.shape == (A,)
    nc = _get_module()
    BL = B // N_CORES

    def pack(mT):  # [rows=128*G, cols] -> [128, G, cols] partition-major
        g = mT.shape[0] // 128
        return np.ascontiguousarray(
            mT.reshape(g, 128, -1).transpose(1, 0, 2)).astype(bf16)

    wencT_np = pack(W_enc.T)
    wdecT_np = pack(W_dec.T)
    vv_np = pack(att_v[:, None])
    eye_np = np.ones((1, 1), np.float32)
    eye128_np = np.eye(128).astype(bf16)

    in_maps = []
    for c in range(N_CORES):
        bs = slice(c * BL, (c + 1) * BL)
        in_maps.append({
            "enc": np.ascontiguousarray(enc_outs[:, bs, :]).reshape(S * BL, HE),
            "wencT": wencT_np,
            "wdecT": wdecT_np,
            "decT": pack(dec_out[bs].T),
            "vv": vv_np,
            "eye1": eye_np,
            "eye128": eye128_np,
        })

    trace = bool(int(os.environ.get("BAHDANAU_TRACE", "0")))
    res = bass_utils.run_bass_kernel_spmd(
        nc, in_maps, core_ids=list(range(N_CORES)), trace=trace)
    _cache["last_result"] = res

    context = np.concatenate([res.results[c]["ctx"] for c in range(N_CORES)], axis=0)
    weights = np.concatenate([res.results[c]["wout"] for c in range(N_CORES)], axis=0)
    return context, weights
